# revision 16
# baseline (speedup 1.0000x reference)
"""Trainium2 Bass kernel for nn_Conv2dMem (bit-slice fake-quantized 3x3 conv).

Math (per image): unfold 3x3/pad1 -> per-row granule (32 along K, C-major)
symmetric int7 fake-quant of activations; per 32x32 block fake-quant of
weights; GEMM; bias.

Strategy (8 cores, batch-parallel, 1 image/core), v3:
  - Weights fake-quantized exactly on host (numpy), fed as fp16 GEMM tiles.
  - Image stays in padded 58-col (C x 58*58) layout end to end; conv = 18
    shifted GEMM accumulations (2 channel-tiles x 9 kernel positions) into
    PSUM; elementwise quantize ops run on full 464-wide (8 rows x 58) f16
    tiles (garbage at pad columns, excluded from GEMM by a strided moving
    AP) so the DVE hits its 2x 16-bit mode.
  - Granule absmax m[g, :] built on-device from image-domain max algebra
    (A=|x|, H2/H3 horizontal, V2/V3 vertical window maxes) + partition-
    strided gather DMAs in 58-layout: whole-partition contiguous copies
    (4 descriptors per gather instead of ~230).
  - Scales r = 63/m, s = m/63 (f16, granule rows) broadcast granule->
    channel rows ONCE per (ct, chunk) via two tiny matmuls (j=0, j=1) +
    PSUM->SBUF f16 copies; per j-step only 4 partition rows change granule
    ((9c+j)//32 crossings), patched by 2 partition-strided row DMAs.
  - Quantize: t16 = x*r (DVE f16), +1536 magic on scalar engine (exact RNE
    int round at f16 writeback), -1536 on DVE, xdq = q*s -> GEMM moving.
"""
import numpy as np
from contextlib import ExitStack

C_IN = 256
N_OUT = 256
H = W = 56
HP = WP = 58
L = H * W                        # 3136
NPW = HP * WP                    # 3364
AW = NPW + 120                   # padded array width (gather windows + slack)
KS = 3
GRAN = 32
NG = (C_IN * KS * KS) // GRAN    # 72 granules
MAXQ = 63.0
C16 = 1536.0                     # f16 magic rounding constant (3 * 2^9)
NCT = 2                          # channel partition tiles (256/128)
NH = 2                           # output-channel halves
ROWS = 8                         # rows per chunk
NCHUNK = H // ROWS               # 7
CHUNK = ROWS * W                 # 448 (compact)
CW = ROWS * WP                   # 464 (58-layout width)
M_CLAMP = 6e-5                   # fp16-safe clamp for zero-granule guard
INV9MOD32 = 25                   # 9 * 25 = 225 = 7*32 + 1

USE_CORR = False                 # fold -1536 into K=72 correction matmuls

ARR_NAMES = ("A", "H2", "H3", "V2", "V3")


# --------------------------------------------------------------------------
# host-side index tables
# --------------------------------------------------------------------------
def granule_terms():
    """For each g' in [0,9): list of (c'', arr, da, db) whose pointwise max
    over terms equals the granule absmax. Padded to 6 terms (repeats)."""
    out = []
    for gp in range(9):
        c_lo, c_hi = (32 * gp) // 9, (32 * gp + 31) // 9
        j0 = 32 * gp - 9 * c_lo
        j1 = 32 * gp + 32 - 9 * c_hi
        terms = []
        if j0 == 0:
            terms.append((c_lo, "V3", 0, 0))
        else:
            dh0, dw0 = divmod(j0, 3)
            terms.append((c_lo, {0: "H3", 1: "H2", 2: "A"}[dw0], dh0, dw0))
            if dh0 == 0:
                terms.append((c_lo, "V2", 1, 0))
            elif dh0 == 1:
                terms.append((c_lo, "H3", 2, 0))
        for c in range(c_lo + 1, c_hi):
            terms.append((c, "V3", 0, 0))
        if j1 == 9:
            terms.append((c_hi, "V3", 0, 0))
        else:
            q, rr = divmod(j1, 3)
            if q == 1:
                terms.append((c_hi, "H3", 0, 0))
            elif q == 2:
                terms.append((c_hi, "V2", 0, 0))
            if rr == 1:
                terms.append((c_hi, "A", q, 0))
            elif rr == 2:
                terms.append((c_hi, "H2", q, 0))
        assert 1 <= len(terms) <= 6, (gp, terms)
        while len(terms) < 6:
            terms.append(terms[0])
        out.append(terms)
    return out


TERMS = granule_terms()


def build_E():
    """E[idx= ct*9+j] : (36, 128) fp16; RJ = E.T @ r[36ct:36ct+36]."""
    E = np.zeros((NCT * 9, 36, 128), np.float16)
    for ct in range(NCT):
        for j in range(9):
            for p in range(128):
                c = 128 * ct + p
                g = (9 * c + j) // 32
                E[ct * 9 + j, g - 36 * ct, p] = 1.0
    return E


def crossings(j):
    """Partitions whose granule increments at step j (same for both ct):
    4 partitions p0+32u; returns (p0, g'(p0, j)) with g' local to 9c+j."""
    p0 = (-INV9MOD32 * j) % 32
    return p0, (9 * p0 + j) // 32


def quantize_weight_host(weight):
    """Exact numpy replica of reference _fake_quant_weight on w2d=(K,N)."""
    w2d = weight.reshape(N_OUT, -1).T.astype(np.float32)      # (2304, 256)
    K, N = w2d.shape
    wg = w2d.reshape(K // 32, 32, N // 32, 32)
    max_abs = np.max(np.abs(wg), axis=(1, 3), keepdims=True)
    scale = (max_abs / np.float32(MAXQ)).astype(np.float32)
    scale = np.where(scale == 0, np.float32(1.0), scale)
    q = np.clip(np.round(wg / scale), -MAXQ, MAXQ)
    deq = (q * scale).astype(np.float32).reshape(K, N)
    return deq


def pack_weights(wdq):
    """(2304, 256) -> W[idx=ct*9+j, nh, p, n] fp16 stationary tiles."""
    Wt = np.zeros((NCT * 9, NH, 128, 128), np.float16)
    for ct in range(NCT):
        for j in range(9):
            rows = (9 * (128 * ct + np.arange(128)) + j)      # (128,)
            for nh in range(NH):
                Wt[ct * 9 + j, nh] = wdq[rows][:, 128 * nh:128 * nh + 128]
    return Wt


def build_wcorr(wdq):
    """(72, 2, 2, 128) f16: hi/lo split of -1536 * per-granule weight sums."""
    wg = wdq.reshape(NG, 32, N_OUT).sum(axis=1).astype(np.float64)  # (72, 256)
    F = (-C16) * wg
    F1 = F.astype(np.float16)
    F2 = (F - F1.astype(np.float64)).astype(np.float16)
    out = np.zeros((NG, 2, NH, 128), np.float16)
    for nh in range(NH):
        out[:, 0, nh] = F1[:, 128 * nh:128 * nh + 128]
        out[:, 1, nh] = F2[:, 128 * nh:128 * nh + 128]
    return out


def pad_image(x):
    """(256,56,56) fp32 -> (2,128,3364) fp16 padded."""
    xp = np.pad(x, ((0, 0), (1, 1), (1, 1))).astype(np.float16)
    return xp.reshape(NCT, 128, NPW)


# --------------------------------------------------------------------------
# numpy model of the device pipeline (for validation in test.py)
# --------------------------------------------------------------------------
def model_core(x, Wt, bias):
    """Bit-accurate-ish numpy model of what the bass kernel computes for one
    image. x: (256,56,56) fp32. Returns (256,56,56) fp32."""
    xp16 = pad_image(x)                                      # (2,128,3364) f16
    xp = xp16.reshape(C_IN, HP, WP)
    A = np.abs(xp)
    H2 = np.maximum(A[:, :, :-1], A[:, :, 1:])               # (C,58,57)
    H3 = np.maximum(H2[:, :, :-1], A[:, :, 2:])              # (C,58,56)
    V2 = np.maximum(H3[:, :-1], H3[:, 1:])                   # (C,57,56)
    V3 = np.maximum(V2[:, :-1], H3[:, 2:])                   # (C,56,56)
    arrs = {"A": A, "H2": H2, "H3": H3, "V2": V2, "V3": V3}

    m = np.zeros((NG, H, W), np.float16)
    for g in range(NG):
        B, gp = divmod(g, 9)
        acc = None
        for (cpp, arr, da, db) in TERMS[gp]:
            c = 32 * B + cpp
            v = arrs[arr][c, da:da + H, db:db + W]
            acc = v if acc is None else np.maximum(acc, v)
        m[g] = acc
    m = np.maximum(m, np.float16(M_CLAMP))

    m32 = m.astype(np.float32)
    rj = (np.float32(MAXQ) / m32).astype(np.float16).astype(np.float32)
    sj = (m32 * np.float32(1.0 / MAXQ)).astype(np.float16).astype(np.float32)

    out = np.zeros((N_OUT, L), np.float32)
    x16 = xp16.reshape(C_IN, HP, WP).astype(np.float32)
    for ct in range(NCT):
        for j in range(9):
            dh, dw = divmod(j, 3)
            cs = np.arange(128 * ct, 128 * ct + 128)
            g = (9 * cs + j) // 32                            # (128,)
            xv = x16[cs, dh:dh + H, dw:dw + W].reshape(128, L)
            t16 = (xv * rj[g].reshape(128, L)).astype(np.float16)
            qb = (t16.astype(np.float32) + np.float32(C16)).astype(np.float16)
            if USE_CORR:
                xdq = (qb.astype(np.float32) * sj[g].reshape(128, L)).astype(np.float16)
            else:
                q = qb.astype(np.float32) - np.float32(C16)   # exact in f16
                xdq = (q * sj[g].reshape(128, L)).astype(np.float16)
            for nh in range(NH):
                Wtile = Wt[ct * 9 + j, nh].astype(np.float32)  # (128c,128n)
                out[128 * nh:128 * nh + 128] += Wtile.T @ xdq.astype(np.float32)
    if USE_CORR:
        wdq = np.zeros((NG * GRAN, N_OUT), np.float32)
        for ct in range(NCT):
            for j in range(9):
                rows = 9 * (128 * ct + np.arange(128)) + j
                for nh in range(NH):
                    wdq[rows, 128 * nh:128 * nh + 128] = Wt[ct * 9 + j, nh]
        F = build_wcorr(wdq)                                  # (72,2,NH,128)
        sg = sj.reshape(NG, L)                                # f32 of f16
        for hl in range(2):
            for nh in range(NH):
                out[128 * nh:128 * nh + 128] += \
                    F[:, hl, nh].astype(np.float32).T @ sg
    out += bias.astype(np.float32)[:, None]
    return out.reshape(N_OUT, H, W)


# --------------------------------------------------------------------------
# bass kernel
# --------------------------------------------------------------------------
_CACHE = {}


def _build_nc():
    import concourse.bass as bass
    import concourse.bacc as bacc
    import concourse.mybir as mybir
    from concourse import tile

    f32, f16 = mybir.dt.float32, mybir.dt.float16
    ALU = mybir.AluOpType
    ACTF = mybir.ActivationFunctionType

    nc = bacc.Bacc("TRN2", target_bir_lowering=False, debug=False)
    xpad_d = nc.dram_tensor("xpad", (NCT, 128, NPW), f16, kind="ExternalInput")
    w_d = nc.dram_tensor("wt", (128, NCT * 9 * NH * 128), f16, kind="ExternalInput")
    e_d = nc.dram_tensor("et", (36, NCT * 9 * 128), f16, kind="ExternalInput")
    b_d = nc.dram_tensor("bias", (128, NH), f32, kind="ExternalInput")
    if USE_CORR:
        wc_d = nc.dram_tensor("wcorr", (NG, 2 * NH * 128), f16, kind="ExternalInput")
    y_d = nc.dram_tensor("y", (NH, 128, L), f32, kind="ExternalOutput")

    es = ExitStack()
    with tile.TileContext(nc) as tc:
        pc = es.enter_context(tc.tile_pool(name="consts", bufs=1))
        pst = es.enter_context(tc.tile_pool(name="staging", bufs=1))
        psb = es.enter_context(tc.tile_pool(name="scalesb", bufs=1))
        pw = es.enter_context(tc.tile_pool(name="work", bufs=1))
        pyo = es.enter_context(tc.tile_pool(name="yout", bufs=2))
        pps = es.enter_context(tc.tile_pool(name="scaleps", bufs=1, space="PSUM"))
        py0 = es.enter_context(tc.tile_pool(name="yps0", bufs=1, space="PSUM"))
        py1 = es.enter_context(tc.tile_pool(name="yps1", bufs=1, space="PSUM"))
        pa = tc.alloc_tile_pool(name="arrays", bufs=1)
        pT = tc.alloc_tile_pool(name="Tstage", bufs=1)

        # ---- load constants -------------------------------------------------
        xp_sb = [pc.tile([128, AW], f16, tag=f"xp{ct}", name=f"xp{ct}")
                 for ct in range(NCT)]
        for ct in range(NCT):
            nc.sync.dma_start(out=xp_sb[ct][:, 0:NPW], in_=xpad_d.ap()[ct])
            nc.vector.memset(xp_sb[ct][:, NPW:AW], 0.0)
        w_sb = pc.tile([128, NCT * 9 * NH * 128], f16, tag="wsb")
        nc.sync.dma_start(out=w_sb[:], in_=w_d.ap())
        e_sb = pc.tile([36, NCT * 9 * 128], f16, tag="esb")
        nc.sync.dma_start(out=e_sb[:], in_=e_d.ap())
        bias_sb = pc.tile([128, NH], f32, tag="bsb")
        nc.sync.dma_start(out=bias_sb[:], in_=b_d.ap())
        bias_c16 = pc.tile([128, 1], f32, tag="bc16")
        nc.vector.memset(bias_c16[:], C16)
        bias_nc16 = pc.tile([128, 1], f32, tag="bnc16")
        nc.vector.memset(bias_nc16[:], -C16)
        if USE_CORR:
            wc_sb = pc.tile([NG, 2 * NH * 128], f16, tag="wcsb")
            nc.sync.dma_start(out=wc_sb[:], in_=wc_d.ap())

        # ---- stage 1: image-domain max algebra (fp16, 58-layout) ------------
        # valid ranges: A [0:AW], H2 [0:AW-1], H3 [0:AW-2],
        #               V2 [0:AW-2-58], V3 [0:AW-2-116]
        arrs = {}
        for name in ARR_NAMES:
            arrs[name] = [pa.tile([128, AW], f16, tag=f"{name}{ct}", name=f"{name}{ct}")
                          for ct in range(NCT)]
        for ct in range(NCT):
            A, H2a, H3a, V2a, V3a = (arrs[n][ct] for n in ARR_NAMES)
            nc.scalar.activation(A[:], xp_sb[ct][:], ACTF.Abs)
            nc.vector.tensor_tensor(H2a[:, 0:AW - 1], A[:, 0:AW - 1],
                                    A[:, 1:AW], op=ALU.max)
            nc.vector.tensor_tensor(H3a[:, 0:AW - 2], H2a[:, 0:AW - 2],
                                    A[:, 2:AW], op=ALU.max)
            nc.vector.tensor_tensor(V2a[:, 0:AW - 2 - HP], H3a[:, 0:AW - 2 - HP],
                                    H3a[:, HP:AW - 2], op=ALU.max)
            nc.vector.tensor_tensor(V3a[:, 0:AW - 2 - 2 * HP], V2a[:, 0:AW - 2 - 2 * HP],
                                    H3a[:, 2 * HP:AW - 2], op=ALU.max)

        # ---- stage 2: granule gather + max tree (58-layout, contiguous) -----
        # T buffers rotate with bufs=3; from round 3 on, a repeated term can
        # be skipped: the stale row (same gp, round i-3) is a valid max member.
        m58 = pst.tile([NG, NPW], f16, tag="m58")
        T_prev = None
        gq = 0
        for i in range(6):
            T_i = pT.tile([NG, NPW], f16, tag="T", name=f"T{i}", bufs=3)
            for gp in range(9):
                cpp, arr, da, db = TERMS[gp][i]
                if i >= 3 and (cpp, arr, da, db) in TERMS[gp][:i]:
                    continue
                off = HP * da + db
                for half in range(NCT):
                    src = arrs[arr][half][cpp:cpp + 97:32, off:off + NPW]
                    dst = T_i[36 * half + gp:36 * half + gp + 28:9]
                    eng = nc.sync if gq % 2 == 0 else nc.scalar
                    gq += 1
                    eng.dma_start(out=dst, in_=src)
            if i == 1:
                nc.vector.tensor_tensor(m58[:], T_prev[:], T_i[:], op=ALU.max)
            elif i > 1:
                nc.vector.tensor_tensor(m58[:], m58[:], T_i[:], op=ALU.max)
            T_prev = T_i
        nc.vector.tensor_scalar(m58[:], m58[:], float(M_CLAMP), None, op0=ALU.max)

        # ---- stage 3: scales (f16, 58-layout); rs58 = [r | s] ---------------
        m32 = pst.tile([NG, NPW], f32, tag="m32")
        rcp32 = pst.tile([NG, NPW], f32, tag="rcp32")
        nc.vector.tensor_copy(m32[:], m58[:])
        nc.vector.reciprocal_approx_fast(out=rcp32[:], in_=m32[:])
        rs58 = psb.tile([NG, 2 * NPW], f16, tag="rs58")
        nc.vector.tensor_scalar(rs58[:, 0:NPW], rcp32[:], float(MAXQ), None,
                                op0=ALU.mult)
        nc.vector.tensor_scalar(rs58[:, NPW:2 * NPW], m58[:], float(1.0 / MAXQ),
                                None, op0=ALU.mult)
        rs58b = psb.tile([36, 2 * NPW], f16, tag="rs58b")
        nc.sync.dma_start(out=rs58b[:], in_=rs58[36:72, :])
        rs_src = [rs58, rs58b]
        rs_lo = [0, 0]    # partition offset within the source tile
        pT.release()
        pa.release()

        # ---- stage 4: main loop (software-pipelined, skew 1) ----------------
        segments = [(ch, ct) for ch in range(NCHUNK) for ct in range(NCT)]
        seg_rsb = {}
        seg_yps = {}

        def emit_base(si):
            """Scale-tile base builds (j=0, j=1) for segment si."""
            ch, ct = segments[si]
            off = HP * ROWS * ch
            src = rs_src[ct]
            rsb = []
            for jb in range(2):
                e_ap = e_sb[:, 128 * (ct * 9 + jb):128 * (ct * 9 + jb) + 128]
                rps = pps.tile([128, CW], f32, tag=f"rps{jb}", name=f"rps{jb}")
                sps = pps.tile([128, CW], f32, tag=f"sps{jb}", name=f"sps{jb}")
                nc.tensor.matmul(rps[:], e_ap, src[0:36, off:off + CW],
                                 start=True, stop=True)
                nc.tensor.matmul(sps[:], e_ap, src[0:36, NPW + off:NPW + off + CW],
                                 start=True, stop=True)
                rt = psb.tile([128, 2 * CW], f16, tag=f"rsb{ct}{jb}",
                              name=f"rsb{ct}{jb}", bufs=2)
                nc.scalar.copy(rt[:, 0:CW], rps[:])
                nc.scalar.copy(rt[:, CW:2 * CW], sps[:])
                rsb.append(rt)
            seg_rsb[si] = rsb

        def round_on_scalar(j):
            return j % 3 != 0

        def emit_fin(si, j, work):
            """Finish unit j of segment si: [DVE round], xdq, matmuls, patch."""
            ch, ct = segments[si]
            off = HP * ROWS * ch
            idx = ct * 9 + j
            buf = seg_rsb[si][j % 2]
            t16, qb = work
            xdq = pw.tile([128, CW], f16, tag="x", bufs=3)
            if USE_CORR:
                if not round_on_scalar(j):
                    nc.vector.tensor_scalar(qb[:], t16[:], C16, None, op0=ALU.add)
                nc.vector.tensor_tensor(xdq[:], qb[:], buf[:, CW:2 * CW],
                                        op=ALU.mult)
            else:
                q16 = pw.tile([128, CW], f16, tag="qs", bufs=3)
                if round_on_scalar(j):
                    # round happened on scalar at emit time; subtract there too
                    nc.scalar.activation(q16[:], qb[:], ACTF.Identity,
                                         bias=bias_nc16[:], scale=1.0)
                else:
                    nc.vector.tensor_scalar(qb[:], t16[:], C16, None, op0=ALU.add)
                    nc.vector.tensor_scalar(q16[:], qb[:], -C16, None, op0=ALU.add)
                nc.vector.tensor_tensor(xdq[:], q16[:], buf[:, CW:2 * CW],
                                        op=ALU.mult)
            xdq_v = xdq.rearrange("p (a b) -> p a b", b=WP)[:, :, 0:W]
            yps = seg_yps[si // NCT]
            for nh in range(NH):
                wsl = w_sb[:, (idx * NH + nh) * 128:(idx * NH + nh + 1) * 128]
                nc.tensor.matmul(yps[nh][:], wsl, xdq_v,
                                 start=(idx == 0),
                                 stop=(idx == NCT * 9 - 1 and not USE_CORR))
            # patch scale tile (buffer j%2) toward scales of j+2
            if j + 2 <= 8:
                src = rs_src[ct]
                eng = nc.sync if ct == 0 else nc.scalar
                for jj in (j + 1, j + 2):
                    p0, gl = crossings(jj)
                    sview = (src.rearrange("p (s c) -> p s c", s=2)
                             [gl:gl + 28:9, :, off:off + CW])
                    dview = (buf.rearrange("p (s c) -> p s c", s=2)
                             [p0:p0 + 97:32])
                    eng.dma_start(out=dview, in_=sview)

        emit_base(0)
        emit_base(1)
        for si, (ch, ct) in enumerate(segments):
            h0 = ROWS * ch
            off = HP * h0
            if ct == 0:
                seg_yps[ch] = [py0.tile([128, CHUNK], f32, tag="y0", name="y0",
                                        bufs=2),
                               py1.tile([128, CHUNK], f32, tag="y1", name="y1",
                                        bufs=2)]
            prev = None
            for j in range(9):
                buf = seg_rsb[si][j % 2]
                dh, dw = divmod(j, 3)
                xv = xp_sb[ct][:, HP * (h0 + dh) + dw:HP * (h0 + dh) + dw + CW]
                t16 = pw.tile([128, CW], f16, tag="t", bufs=3)
                qb = pw.tile([128, CW], f16, tag="q", bufs=3)
                nc.vector.tensor_tensor(t16[:], xv, buf[:, 0:CW], op=ALU.mult)
                if round_on_scalar(j):
                    nc.scalar.activation(qb[:], t16[:], ACTF.Identity,
                                         bias=bias_c16[:], scale=1.0)
                if j >= 1:
                    emit_fin(si, j - 1, prev)
                if j == 4 and si + 2 < len(segments):
                    emit_base(si + 2)
                prev = (t16, qb)
            emit_fin(si, 8, prev)
            if ct == 1:
                lsl = slice(CHUNK * ch, CHUNK * (ch + 1))
                yps = seg_yps[ch]
                if USE_CORR:
                    sv = (rs58.rearrange("p (s c) -> p s c", s=2)
                          [:, 1, off:off + CW]
                          .rearrange("p (a b) -> p a b", b=WP)[:, :, 0:W])
                    for hl in range(2):
                        for nh in range(NH):
                            wc_ap = wc_sb[:, (hl * NH + nh) * 128:
                                          (hl * NH + nh + 1) * 128]
                            nc.tensor.matmul(yps[nh][:], wc_ap, sv,
                                             start=False, stop=(hl == 1))
                for nh in range(NH):
                    ysb = pyo.tile([128, CHUNK], f32, tag=f"ysb{nh}")
                    nc.scalar.activation(ysb[:], yps[nh][:], ACTF.Identity,
                                         bias=bias_sb[:, nh:nh + 1], scale=1.0)
                    nc.sync.dma_start(out=y_d.ap()[nh, :, lsl], in_=ysb[:])
        es.close()
    nc.compile()
    return nc


def build_inmaps(input, weight, bias):
    """FULL inputs -> list of 8 per-core input dicts."""
    input = np.asarray(input, np.float32)
    weight = np.asarray(weight, np.float32)
    bias = np.asarray(bias, np.float32)
    wdq = quantize_weight_host(weight)
    Wt = np.ascontiguousarray(np.transpose(pack_weights(wdq), (2, 0, 1, 3))
                              ).reshape(128, NCT * 9 * NH * 128)
    E = np.ascontiguousarray(np.transpose(build_E(), (1, 0, 2))).reshape(36, NCT * 9 * 128)
    b = np.ascontiguousarray(bias.reshape(NH, 128).T).astype(np.float32)
    base = {"wt": Wt, "et": E, "bias": b}
    if USE_CORR:
        wdq16 = wdq.astype(np.float16).astype(np.float64)
        base["wcorr"] = build_wcorr(wdq16).reshape(NG, 2 * NH * 128)
    return [dict(base, xpad=pad_image(input[bi])) for bi in range(input.shape[0])]


def kernel(input, weight, bias):
    input = np.asarray(input, np.float32)
    B = input.shape[0]
    assert B == 8 and input.shape[1:] == (C_IN, H, W)

    from concourse import bass_utils

    if "nc" not in _CACHE:
        _CACHE["nc"] = _build_nc()
    nc = _CACHE["nc"]

    in_maps = build_inmaps(input, weight, bias)
    res = bass_utils.run_bass_kernel_spmd(nc, in_maps, core_ids=list(range(B)))
    out = np.stack([r["y"].reshape(N_OUT, H, W) for r in res.results])
    return out.astype(np.float32)


if __name__ == "__main__":
    pass


# revision 22
# speedup vs baseline: 1.4144x; 1.4144x over previous
"""Trainium2 Bass kernel for nn_Conv2dMem (bit-slice fake-quantized 3x3 conv).

Math (per image): unfold 3x3/pad1 -> per-row granule (32 along K, C-major)
symmetric int7 fake-quant of activations; per 32x32 block fake-quant of
weights; GEMM; bias.

Strategy (8 cores, batch-parallel, 1 image/core), v3:
  - Weights fake-quantized exactly on host (numpy), fed as fp16 GEMM tiles.
  - Image stays in padded 58-col (C x 58*58) layout end to end; conv = 18
    shifted GEMM accumulations (2 channel-tiles x 9 kernel positions) into
    PSUM; elementwise quantize ops run on full 464-wide (8 rows x 58) f16
    tiles (garbage at pad columns, excluded from GEMM by a strided moving
    AP) so the DVE hits its 2x 16-bit mode.
  - Granule absmax m[g, :] built on-device from image-domain max algebra
    (A=|x|, H2/H3 horizontal, V2/V3 vertical window maxes) + partition-
    strided gather DMAs in 58-layout: whole-partition contiguous copies
    (4 descriptors per gather instead of ~230).
  - Scales r = 63/m, s = m/63 (f16, granule rows) broadcast granule->
    channel rows ONCE per (ct, chunk) via two tiny matmuls (j=0, j=1) +
    PSUM->SBUF f16 copies; per j-step only 4 partition rows change granule
    ((9c+j)//32 crossings), patched by 2 partition-strided row DMAs.
  - Quantize: t16 = x*r (DVE f16), +1536 magic on scalar engine (exact RNE
    int round at f16 writeback), -1536 on DVE, xdq = q*s -> GEMM moving.
"""
import numpy as np
from contextlib import ExitStack

C_IN = 256
N_OUT = 256
H = W = 56
HP = WP = 58
L = H * W                        # 3136
NPW = HP * WP                    # 3364
AW = NPW + 120                   # padded array width (gather windows + slack)
KS = 3
GRAN = 32
NG = (C_IN * KS * KS) // GRAN    # 72 granules
MAXQ = 63.0
C16 = 1536.0                     # f16 magic rounding constant (3 * 2^9)
NCT = 2                          # channel partition tiles (256/128)
NH = 2                           # output-channel halves
ROWS = 8                         # rows per chunk
NCHUNK = H // ROWS               # 7
CHUNK = ROWS * W                 # 448 (compact)
CW = ROWS * WP                   # 464 (58-layout width)
M_CLAMP = 6e-5                   # fp16-safe clamp for zero-granule guard
INV9MOD32 = 25                   # 9 * 25 = 225 = 7*32 + 1

USE_CORR = False                 # fold -1536 into K=72 correction matmuls

ARR_NAMES = ("A", "H2", "H3", "V2", "V3")


# --------------------------------------------------------------------------
# host-side index tables
# --------------------------------------------------------------------------
def granule_terms():
    """For each g' in [0,9): list of (c'', arr, da, db) whose pointwise max
    over terms equals the granule absmax. Padded to 6 terms (repeats)."""
    out = []
    for gp in range(9):
        c_lo, c_hi = (32 * gp) // 9, (32 * gp + 31) // 9
        j0 = 32 * gp - 9 * c_lo
        j1 = 32 * gp + 32 - 9 * c_hi
        terms = []
        if j0 == 0:
            terms.append((c_lo, "V3", 0, 0))
        else:
            dh0, dw0 = divmod(j0, 3)
            terms.append((c_lo, {0: "H3", 1: "H2", 2: "A"}[dw0], dh0, dw0))
            if dh0 == 0:
                terms.append((c_lo, "V2", 1, 0))
            elif dh0 == 1:
                terms.append((c_lo, "H3", 2, 0))
        for c in range(c_lo + 1, c_hi):
            terms.append((c, "V3", 0, 0))
        if j1 == 9:
            terms.append((c_hi, "V3", 0, 0))
        else:
            q, rr = divmod(j1, 3)
            if q == 1:
                terms.append((c_hi, "H3", 0, 0))
            elif q == 2:
                terms.append((c_hi, "V2", 0, 0))
            if rr == 1:
                terms.append((c_hi, "A", q, 0))
            elif rr == 2:
                terms.append((c_hi, "H2", q, 0))
        assert 1 <= len(terms) <= 6, (gp, terms)
        while len(terms) < 6:
            terms.append(terms[0])
        out.append(terms)
    return out


TERMS = granule_terms()


def build_E():
    """E[idx= ct*9+j] : (36, 128) fp16; RJ = E.T @ r[36ct:36ct+36]."""
    E = np.zeros((NCT * 9, 36, 128), np.float16)
    for ct in range(NCT):
        for j in range(9):
            for p in range(128):
                c = 128 * ct + p
                g = (9 * c + j) // 32
                E[ct * 9 + j, g - 36 * ct, p] = 1.0
    return E


def crossings(j):
    """Partitions whose granule increments at step j (same for both ct):
    4 partitions p0+32u; returns (p0, g'(p0, j)) with g' local to 9c+j."""
    p0 = (-INV9MOD32 * j) % 32
    return p0, (9 * p0 + j) // 32


def quantize_weight_host(weight):
    """Exact numpy replica of reference _fake_quant_weight on w2d=(K,N)."""
    w2d = weight.reshape(N_OUT, -1).T.astype(np.float32)      # (2304, 256)
    K, N = w2d.shape
    wg = w2d.reshape(K // 32, 32, N // 32, 32)
    max_abs = np.max(np.abs(wg), axis=(1, 3), keepdims=True)
    scale = (max_abs / np.float32(MAXQ)).astype(np.float32)
    scale = np.where(scale == 0, np.float32(1.0), scale)
    q = np.clip(np.round(wg / scale), -MAXQ, MAXQ)
    deq = (q * scale).astype(np.float32).reshape(K, N)
    return deq


def pack_weights(wdq):
    """(2304, 256) -> W[idx=ct*9+j, nh, p, n] fp16 stationary tiles."""
    Wt = np.zeros((NCT * 9, NH, 128, 128), np.float16)
    for ct in range(NCT):
        for j in range(9):
            rows = (9 * (128 * ct + np.arange(128)) + j)      # (128,)
            for nh in range(NH):
                Wt[ct * 9 + j, nh] = wdq[rows][:, 128 * nh:128 * nh + 128]
    return Wt


def build_wcorr(wdq):
    """(72, 2, 2, 128) f16: hi/lo split of -1536 * per-granule weight sums."""
    wg = wdq.reshape(NG, 32, N_OUT).sum(axis=1).astype(np.float64)  # (72, 256)
    F = (-C16) * wg
    F1 = F.astype(np.float16)
    F2 = (F - F1.astype(np.float64)).astype(np.float16)
    out = np.zeros((NG, 2, NH, 128), np.float16)
    for nh in range(NH):
        out[:, 0, nh] = F1[:, 128 * nh:128 * nh + 128]
        out[:, 1, nh] = F2[:, 128 * nh:128 * nh + 128]
    return out


def pad_image(x):
    """(256,56,56) fp32 -> (2,128,3364) fp16 padded."""
    xp = np.pad(x, ((0, 0), (1, 1), (1, 1))).astype(np.float16)
    return xp.reshape(NCT, 128, NPW)


# --------------------------------------------------------------------------
# numpy model of the device pipeline (for validation in test.py)
# --------------------------------------------------------------------------
def model_core(x, Wt, bias):
    """Bit-accurate-ish numpy model of what the bass kernel computes for one
    image. x: (256,56,56) fp32. Returns (256,56,56) fp32."""
    xp16 = pad_image(x)                                      # (2,128,3364) f16
    xp = xp16.reshape(C_IN, HP, WP)
    A = np.abs(xp)
    H2 = np.maximum(A[:, :, :-1], A[:, :, 1:])               # (C,58,57)
    H3 = np.maximum(H2[:, :, :-1], A[:, :, 2:])              # (C,58,56)
    V2 = np.maximum(H3[:, :-1], H3[:, 1:])                   # (C,57,56)
    V3 = np.maximum(V2[:, :-1], H3[:, 2:])                   # (C,56,56)
    arrs = {"A": A, "H2": H2, "H3": H3, "V2": V2, "V3": V3}

    m = np.zeros((NG, H, W), np.float16)
    for g in range(NG):
        B, gp = divmod(g, 9)
        acc = None
        for (cpp, arr, da, db) in TERMS[gp]:
            c = 32 * B + cpp
            v = arrs[arr][c, da:da + H, db:db + W]
            acc = v if acc is None else np.maximum(acc, v)
        m[g] = acc
    m = np.maximum(m, np.float16(M_CLAMP))

    m32 = m.astype(np.float32)
    rj = (np.float32(MAXQ) / m32).astype(np.float16).astype(np.float32)
    sj = (m32 * np.float32(1.0 / MAXQ)).astype(np.float16).astype(np.float32)

    out = np.zeros((N_OUT, L), np.float32)
    x16 = xp16.reshape(C_IN, HP, WP).astype(np.float32)
    for ct in range(NCT):
        for j in range(9):
            dh, dw = divmod(j, 3)
            cs = np.arange(128 * ct, 128 * ct + 128)
            g = (9 * cs + j) // 32                            # (128,)
            xv = x16[cs, dh:dh + H, dw:dw + W].reshape(128, L)
            t16 = (xv * rj[g].reshape(128, L)).astype(np.float16)
            q = np.rint(t16.astype(np.float64)).astype(np.float32)  # i16 cast
            xdq = (q * sj[g].reshape(128, L)).astype(np.float16)
            for nh in range(NH):
                Wtile = Wt[ct * 9 + j, nh].astype(np.float32)  # (128c,128n)
                out[128 * nh:128 * nh + 128] += Wtile.T @ xdq.astype(np.float32)
    out += bias.astype(np.float32)[:, None]
    return out.reshape(N_OUT, H, W)


# --------------------------------------------------------------------------
# bass kernel
# --------------------------------------------------------------------------
_CACHE = {}


def _build_nc():
    import concourse.bass as bass
    import concourse.bacc as bacc
    import concourse.mybir as mybir
    from concourse import tile

    f32, f16 = mybir.dt.float32, mybir.dt.float16
    i16 = mybir.dt.int16
    ALU = mybir.AluOpType
    ACTF = mybir.ActivationFunctionType

    nc = bacc.Bacc("TRN2", target_bir_lowering=False, debug=False)
    xpad_d = nc.dram_tensor("xpad", (NCT, 128, NPW), f16, kind="ExternalInput")
    w_d = nc.dram_tensor("wt", (128, NCT * 9 * NH * 128), f16, kind="ExternalInput")
    e_d = nc.dram_tensor("et", (36, NCT * 9 * 128), f16, kind="ExternalInput")
    b_d = nc.dram_tensor("bias", (128, NH), f32, kind="ExternalInput")
    if USE_CORR:
        wc_d = nc.dram_tensor("wcorr", (NG, 2 * NH * 128), f16, kind="ExternalInput")
    y_d = nc.dram_tensor("y", (NH, 128, L), f32, kind="ExternalOutput")

    es = ExitStack()
    with tile.TileContext(nc) as tc:
        pc = es.enter_context(tc.tile_pool(name="consts", bufs=1))
        pst = es.enter_context(tc.tile_pool(name="staging", bufs=1))
        psb = es.enter_context(tc.tile_pool(name="scalesb", bufs=1))
        pw = es.enter_context(tc.tile_pool(name="work", bufs=1))
        pyo = es.enter_context(tc.tile_pool(name="yout", bufs=2))
        pps = es.enter_context(tc.tile_pool(name="scaleps", bufs=1, space="PSUM"))
        py0 = es.enter_context(tc.tile_pool(name="yps0", bufs=1, space="PSUM"))
        py1 = es.enter_context(tc.tile_pool(name="yps1", bufs=1, space="PSUM"))
        pa = tc.alloc_tile_pool(name="arrays", bufs=1)
        pT = tc.alloc_tile_pool(name="Tstage", bufs=1)

        # ---- load constants -------------------------------------------------
        xp_sb = [pc.tile([128, AW], f16, tag=f"xp{ct}", name=f"xp{ct}")
                 for ct in range(NCT)]
        nc.sync.dma_start(out=xp_sb[0][:, 0:NPW], in_=xpad_d.ap()[0])
        nc.scalar.dma_start(out=xp_sb[1][:, 0:NPW], in_=xpad_d.ap()[1])
        for ct in range(NCT):
            nc.vector.memset(xp_sb[ct][:, NPW:AW], 0.0)
        e_sb = pc.tile([36, NCT * 9 * 128], f16, tag="esb")
        nc.sync.dma_start(out=e_sb[:], in_=e_d.ap())
        bias_sb = pc.tile([128, NH], f32, tag="bsb")
        nc.sync.dma_start(out=bias_sb[:], in_=b_d.ap())
        w_sb = pc.tile([128, NCT * 9 * NH * 128], f16, tag="wsb")
        nc.scalar.dma_start(out=w_sb[:], in_=w_d.ap())

        # ---- stage 1: image-domain max algebra (fp16, 58-layout) ------------
        # valid ranges: A [0:AW], H2 [0:AW-1], H3 [0:AW-2],
        #               V2 [0:AW-2-58], V3 [0:AW-2-116]
        arrs = {}
        for name in ARR_NAMES:
            arrs[name] = [pa.tile([128, AW], f16, tag=f"{name}{ct}", name=f"{name}{ct}")
                          for ct in range(NCT)]
        for ct in range(NCT):
            A, H2a, H3a, V2a, V3a = (arrs[n][ct] for n in ARR_NAMES)
            nc.scalar.activation(A[:], xp_sb[ct][:], ACTF.Abs)
            nc.vector.tensor_tensor(H2a[:, 0:AW - 1], A[:, 0:AW - 1],
                                    A[:, 1:AW], op=ALU.max)
            nc.vector.tensor_tensor(H3a[:, 0:AW - 2], H2a[:, 0:AW - 2],
                                    A[:, 2:AW], op=ALU.max)
            nc.vector.tensor_tensor(V2a[:, 0:AW - 2 - HP], H3a[:, 0:AW - 2 - HP],
                                    H3a[:, HP:AW - 2], op=ALU.max)
            nc.vector.tensor_tensor(V3a[:, 0:AW - 2 - 2 * HP], V2a[:, 0:AW - 2 - 2 * HP],
                                    H3a[:, 2 * HP:AW - 2], op=ALU.max)

        # ---- stage 2: granule gather + max tree (58-layout, contiguous) -----
        # T buffers rotate with bufs=3; from round 3 on, a repeated term can
        # be skipped: the stale row (same gp, round i-3) is a valid max member.
        m58 = pst.tile([NG, NPW], f16, tag="m58")
        T_prev = None
        gq = 0
        for i in range(6):
            T_i = pT.tile([NG, NPW], f16, tag="T", name=f"T{i}", bufs=3)
            for gp in range(9):
                cpp, arr, da, db = TERMS[gp][i]
                if i >= 3 and (cpp, arr, da, db) in TERMS[gp][:i]:
                    continue
                off = HP * da + db
                for half in range(NCT):
                    src = arrs[arr][half][cpp:cpp + 97:32, off:off + NPW]
                    dst = T_i[36 * half + gp:36 * half + gp + 28:9]
                    eng = nc.sync if gq % 2 == 0 else nc.scalar
                    gq += 1
                    eng.dma_start(out=dst, in_=src)
            if i == 1:
                nc.vector.tensor_tensor(m58[:], T_prev[:], T_i[:], op=ALU.max)
            elif i > 1:
                nc.vector.tensor_tensor(m58[:], m58[:], T_i[:], op=ALU.max)
            T_prev = T_i
        nc.vector.tensor_scalar(m58[:], m58[:], float(M_CLAMP), None, op0=ALU.max)

        # ---- stage 3: scales (f16, 58-layout); rs58 = [r | s] ---------------
        pT.release()
        pa.release()
        pf = tc.alloc_tile_pool(name="f32tmp", bufs=1)
        m32 = pf.tile([NG, NPW], f32, tag="m32")
        rcp32 = pf.tile([NG, NPW], f32, tag="rcp32")
        nc.vector.tensor_copy(m32[:], m58[:])
        nc.vector.reciprocal_approx_fast(out=rcp32[:], in_=m32[:])
        rs58 = psb.tile([NG, 2 * NPW], f16, tag="rs58")
        nc.vector.tensor_scalar(rs58[:, 0:NPW], rcp32[:], float(MAXQ), None,
                                op0=ALU.mult)
        nc.vector.tensor_scalar(rs58[:, NPW:2 * NPW], m58[:], float(1.0 / MAXQ),
                                None, op0=ALU.mult)
        rs58b = psb.tile([36, 2 * NPW], f16, tag="rs58b")
        nc.sync.dma_start(out=rs58b[:], in_=rs58[36:72, :])
        rs_src = [rs58, rs58b]
        pf.release()

        # ---- stage 4: main loop (software-pipelined, skew 1) ----------------
        segments = [(ch, ct) for ch in range(NCHUNK) for ct in range(NCT)]
        seg_rsb = {}
        seg_yps = {}

        NB = 3                      # rotating scale buffers per segment
        patch_rr = [0]              # round-robin dma issue engine

        def emit_base(si):
            """Scale-tile base builds (j=0..NB-1) for segment si."""
            ch, ct = segments[si]
            off = HP * ROWS * ch
            src = rs_src[ct]
            rsb = []
            for jb in range(NB):
                e_ap = e_sb[:, 128 * (ct * 9 + jb):128 * (ct * 9 + jb) + 128]
                rps = pps.tile([128, CW], f32, tag="rps", name="rps", bufs=2)
                sps = pps.tile([128, CW], f32, tag="sps", name="sps", bufs=2)
                nc.tensor.matmul(rps[:], e_ap, src[0:36, off:off + CW],
                                 start=True, stop=True)
                nc.tensor.matmul(sps[:], e_ap, src[0:36, NPW + off:NPW + off + CW],
                                 start=True, stop=True)
                rt = psb.tile([128, 2 * CW], f16, tag=f"rsb{ct}{jb}",
                              name=f"rsb{ct}{jb}", bufs=2)
                nc.scalar.copy(rt[:, 0:CW], rps[:])
                nc.scalar.copy(rt[:, CW:2 * CW], sps[:])
                rsb.append(rt)
            seg_rsb[si] = rsb

        def emit_unit(si, j):
            """Unit j of segment si: quantize chain, matmuls, patch."""
            ch, ct = segments[si]
            h0 = ROWS * ch
            off = HP * h0
            idx = ct * 9 + j
            buf = seg_rsb[si][j % NB]
            dh, dw = divmod(j, 3)
            xv = xp_sb[ct][:, HP * (h0 + dh) + dw:HP * (h0 + dh) + dw + CW]
            t16 = pw.tile([128, CW], f16, tag="t", bufs=3)
            qi = pw.tile([128, CW], i16, tag="qi", bufs=3)
            xdq = pw.tile([128, CW], f16, tag="x", bufs=3)
            nc.vector.tensor_tensor(t16[:], xv, buf[:, 0:CW], op=ALU.mult)
            nc.vector.tensor_copy(qi[:], t16[:])       # RNE round via i16 cast
            nc.vector.tensor_tensor(xdq[:], qi[:], buf[:, CW:2 * CW],
                                    op=ALU.mult)
            xdq_v = xdq.rearrange("p (a b) -> p a b", b=WP)[:, :, 0:W]
            yps = seg_yps[si // NCT]
            for nh in range(NH):
                wsl = w_sb[:, (idx * NH + nh) * 128:(idx * NH + nh + 1) * 128]
                nc.tensor.matmul(yps[nh][:], wsl, xdq_v,
                                 start=(idx == 0), stop=(idx == NCT * 9 - 1))
            # patch scale buffer (j % NB) toward scales of j+NB
            if j + NB <= 8:
                src = rs_src[ct]
                for jj in range(j + 1, j + NB + 1):
                    eng = nc.sync if patch_rr[0] % 2 == 0 else nc.scalar
                    patch_rr[0] += 1
                    p0, gl = crossings(jj)
                    sview = (src.rearrange("p (s c) -> p s c", s=2)
                             [gl:gl + 28:9, :, off:off + CW])
                    dview = (buf.rearrange("p (s c) -> p s c", s=2)
                             [p0:p0 + 97:32])
                    eng.dma_start(out=dview, in_=sview)

        emit_base(0)
        emit_base(1)
        for si, (ch, ct) in enumerate(segments):
            if ct == 0:
                seg_yps[ch] = [py0.tile([128, CHUNK], f32, tag="y0", name="y0",
                                        bufs=2),
                               py1.tile([128, CHUNK], f32, tag="y1", name="y1",
                                        bufs=2)]
            for j in range(9):
                emit_unit(si, j)
                if j == 4 and si + 2 < len(segments):
                    emit_base(si + 2)
            if ct == 1:
                lsl = slice(CHUNK * ch, CHUNK * (ch + 1))
                yps = seg_yps[ch]
                for nh in range(NH):
                    ysb = pyo.tile([128, CHUNK], f32, tag=f"ysb{nh}")
                    nc.scalar.activation(ysb[:], yps[nh][:], ACTF.Identity,
                                         bias=bias_sb[:, nh:nh + 1], scale=1.0)
                    nc.sync.dma_start(out=y_d.ap()[nh, :, lsl], in_=ysb[:])
        es.close()
    nc.compile()
    return nc


def build_inmaps(input, weight, bias):
    """FULL inputs -> list of 8 per-core input dicts."""
    input = np.asarray(input, np.float32)
    weight = np.asarray(weight, np.float32)
    bias = np.asarray(bias, np.float32)
    wdq = quantize_weight_host(weight)
    Wt = np.ascontiguousarray(np.transpose(pack_weights(wdq), (2, 0, 1, 3))
                              ).reshape(128, NCT * 9 * NH * 128)
    E = np.ascontiguousarray(np.transpose(build_E(), (1, 0, 2))).reshape(36, NCT * 9 * 128)
    b = np.ascontiguousarray(bias.reshape(NH, 128).T).astype(np.float32)
    base = {"wt": Wt, "et": E, "bias": b}
    if USE_CORR:
        wdq16 = wdq.astype(np.float16).astype(np.float64)
        base["wcorr"] = build_wcorr(wdq16).reshape(NG, 2 * NH * 128)
    return [dict(base, xpad=pad_image(input[bi])) for bi in range(input.shape[0])]


def kernel(input, weight, bias):
    input = np.asarray(input, np.float32)
    B = input.shape[0]
    assert B == 8 and input.shape[1:] == (C_IN, H, W)

    from concourse import bass_utils

    if "nc" not in _CACHE:
        _CACHE["nc"] = _build_nc()
    nc = _CACHE["nc"]

    in_maps = build_inmaps(input, weight, bias)
    res = bass_utils.run_bass_kernel_spmd(nc, in_maps, core_ids=list(range(B)))
    out = np.stack([r["y"].reshape(N_OUT, H, W) for r in res.results])
    return out.astype(np.float32)


if __name__ == "__main__":
    pass


# revision 32
# speedup vs baseline: 1.4222x; 1.0056x over previous
"""Trainium2 Bass kernel for nn_Conv2dMem (bit-slice fake-quantized 3x3 conv).

Math (per image): unfold 3x3/pad1 -> per-row granule (32 along K, C-major)
symmetric int7 fake-quant of activations; per 32x32 block fake-quant of
weights; GEMM; bias.

Strategy (8 cores, batch-parallel, 1 image/core), v3:
  - Weights fake-quantized exactly on host (numpy), fed as fp16 GEMM tiles.
  - Image stays in padded 58-col (C x 58*58) layout end to end; conv = 18
    shifted GEMM accumulations (2 channel-tiles x 9 kernel positions) into
    PSUM; elementwise quantize ops run on full 464-wide (8 rows x 58) f16
    tiles (garbage at pad columns, excluded from GEMM by a strided moving
    AP) so the DVE hits its 2x 16-bit mode.
  - Granule absmax m[g, :] built on-device from image-domain max algebra
    (A=|x|, H2/H3 horizontal, V2/V3 vertical window maxes) + partition-
    strided gather DMAs in 58-layout: whole-partition contiguous copies
    (4 descriptors per gather instead of ~230).
  - Scales r = 63/m, s = m/63 (f16, granule rows) broadcast granule->
    channel rows ONCE per (ct, chunk) via two tiny matmuls (j=0, j=1) +
    PSUM->SBUF f16 copies; per j-step only 4 partition rows change granule
    ((9c+j)//32 crossings), patched by 2 partition-strided row DMAs.
  - Quantize: t16 = x*r (DVE f16), +1536 magic on scalar engine (exact RNE
    int round at f16 writeback), -1536 on DVE, xdq = q*s -> GEMM moving.
"""
import numpy as np
from contextlib import ExitStack

C_IN = 256
N_OUT = 256
H = W = 56
HP = WP = 58
L = H * W                        # 3136
NPW = HP * WP                    # 3364
AW = NPW + 120                   # padded array width (gather windows + slack)
KS = 3
GRAN = 32
NG = (C_IN * KS * KS) // GRAN    # 72 granules
MAXQ = 63.0
C16 = 1536.0                     # f16 magic rounding constant (3 * 2^9)
NCT = 2                          # channel partition tiles (256/128)
NH = 2                           # output-channel halves
ROWS = 8                         # rows per chunk
NCHUNK = H // ROWS               # 7
CHUNK = ROWS * W                 # 448 (compact)
CW = ROWS * WP                   # 464 (58-layout width)
M_CLAMP = 6e-5                   # fp16-safe clamp for zero-granule guard
INV9MOD32 = 25                   # 9 * 25 = 225 = 7*32 + 1

USE_CORR = False                 # fold -1536 into K=72 correction matmuls

ARR_NAMES = ("A", "H2", "H3", "V2", "V3")


# --------------------------------------------------------------------------
# host-side index tables
# --------------------------------------------------------------------------
def granule_terms():
    """For each g' in [0,9): list of (c'', arr, da, db) whose pointwise max
    over terms equals the granule absmax. Padded to 6 terms (repeats)."""
    out = []
    for gp in range(9):
        c_lo, c_hi = (32 * gp) // 9, (32 * gp + 31) // 9
        j0 = 32 * gp - 9 * c_lo
        j1 = 32 * gp + 32 - 9 * c_hi
        terms = []
        if j0 == 0:
            terms.append((c_lo, "V3", 0, 0))
        else:
            dh0, dw0 = divmod(j0, 3)
            terms.append((c_lo, {0: "H3", 1: "H2", 2: "A"}[dw0], dh0, dw0))
            if dh0 == 0:
                terms.append((c_lo, "V2", 1, 0))
            elif dh0 == 1:
                terms.append((c_lo, "H3", 2, 0))
        for c in range(c_lo + 1, c_hi):
            terms.append((c, "V3", 0, 0))
        if j1 == 9:
            terms.append((c_hi, "V3", 0, 0))
        else:
            q, rr = divmod(j1, 3)
            if q == 1:
                terms.append((c_hi, "H3", 0, 0))
            elif q == 2:
                terms.append((c_hi, "V2", 0, 0))
            if rr == 1:
                terms.append((c_hi, "A", q, 0))
            elif rr == 2:
                terms.append((c_hi, "H2", q, 0))
        assert 1 <= len(terms) <= 6, (gp, terms)
        while len(terms) < 6:
            terms.append(terms[0])
        out.append(terms)
    return out


TERMS = granule_terms()


def build_E():
    """E[idx= ct*9+j] : (36, 128) fp16; RJ = E.T @ r[36ct:36ct+36]."""
    E = np.zeros((NCT * 9, 36, 128), np.float16)
    for ct in range(NCT):
        for j in range(9):
            for p in range(128):
                c = 128 * ct + p
                g = (9 * c + j) // 32
                E[ct * 9 + j, g - 36 * ct, p] = 1.0
    return E


def crossings(j):
    """Partitions whose granule increments at step j (same for both ct):
    4 partitions p0+32u; returns (p0, g'(p0, j)) with g' local to 9c+j."""
    p0 = (-INV9MOD32 * j) % 32
    return p0, (9 * p0 + j) // 32


def quantize_weight_host(weight):
    """Exact numpy replica of reference _fake_quant_weight on w2d=(K,N)."""
    w2d = weight.reshape(N_OUT, -1).T.astype(np.float32)      # (2304, 256)
    K, N = w2d.shape
    wg = w2d.reshape(K // 32, 32, N // 32, 32)
    max_abs = np.max(np.abs(wg), axis=(1, 3), keepdims=True)
    scale = (max_abs / np.float32(MAXQ)).astype(np.float32)
    scale = np.where(scale == 0, np.float32(1.0), scale)
    q = np.clip(np.round(wg / scale), -MAXQ, MAXQ)
    deq = (q * scale).astype(np.float32).reshape(K, N)
    return deq


def pack_weights(wdq):
    """(2304, 256) -> W[idx=ct*9+j, nh, p, n] fp16 stationary tiles."""
    Wt = np.zeros((NCT * 9, NH, 128, 128), np.float16)
    for ct in range(NCT):
        for j in range(9):
            rows = (9 * (128 * ct + np.arange(128)) + j)      # (128,)
            for nh in range(NH):
                Wt[ct * 9 + j, nh] = wdq[rows][:, 128 * nh:128 * nh + 128]
    return Wt


def build_wcorr(wdq):
    """(72, 2, 2, 128) f16: hi/lo split of -1536 * per-granule weight sums."""
    wg = wdq.reshape(NG, 32, N_OUT).sum(axis=1).astype(np.float64)  # (72, 256)
    F = (-C16) * wg
    F1 = F.astype(np.float16)
    F2 = (F - F1.astype(np.float64)).astype(np.float16)
    out = np.zeros((NG, 2, NH, 128), np.float16)
    for nh in range(NH):
        out[:, 0, nh] = F1[:, 128 * nh:128 * nh + 128]
        out[:, 1, nh] = F2[:, 128 * nh:128 * nh + 128]
    return out


def pad_image(x):
    """(256,56,56) fp32 -> (2,128,3364) fp16 padded."""
    xp = np.pad(x, ((0, 0), (1, 1), (1, 1))).astype(np.float16)
    return xp.reshape(NCT, 128, NPW)


# --------------------------------------------------------------------------
# numpy model of the device pipeline (for validation in test.py)
# --------------------------------------------------------------------------
def model_core(x, Wt, bias):
    """Bit-accurate-ish numpy model of what the bass kernel computes for one
    image. x: (256,56,56) fp32. Returns (256,56,56) fp32."""
    xp16 = pad_image(x)                                      # (2,128,3364) f16
    xp = xp16.reshape(C_IN, HP, WP)
    A = np.abs(xp)
    H2 = np.maximum(A[:, :, :-1], A[:, :, 1:])               # (C,58,57)
    H3 = np.maximum(H2[:, :, :-1], A[:, :, 2:])              # (C,58,56)
    V2 = np.maximum(H3[:, :-1], H3[:, 1:])                   # (C,57,56)
    V3 = np.maximum(V2[:, :-1], H3[:, 2:])                   # (C,56,56)
    arrs = {"A": A, "H2": H2, "H3": H3, "V2": V2, "V3": V3}

    m = np.zeros((NG, H, W), np.float16)
    for g in range(NG):
        B, gp = divmod(g, 9)
        acc = None
        for (cpp, arr, da, db) in TERMS[gp]:
            c = 32 * B + cpp
            v = arrs[arr][c, da:da + H, db:db + W]
            acc = v if acc is None else np.maximum(acc, v)
        m[g] = acc
    m = np.maximum(m, np.float16(M_CLAMP))

    m32 = m.astype(np.float32)
    rj = (np.float32(MAXQ) / m32).astype(np.float16).astype(np.float32)
    sj = (m32 * np.float32(1.0 / MAXQ)).astype(np.float16).astype(np.float32)

    out = np.zeros((N_OUT, L), np.float32)
    x16 = xp16.reshape(C_IN, HP, WP).astype(np.float32)
    for ct in range(NCT):
        for j in range(9):
            dh, dw = divmod(j, 3)
            cs = np.arange(128 * ct, 128 * ct + 128)
            g = (9 * cs + j) // 32                            # (128,)
            xv = x16[cs, dh:dh + H, dw:dw + W].reshape(128, L)
            t16 = (xv * rj[g].reshape(128, L)).astype(np.float16)
            q = np.rint(t16.astype(np.float64)).astype(np.float32)  # i16 cast
            xdq = (q * sj[g].reshape(128, L)).astype(np.float16)
            for nh in range(NH):
                Wtile = Wt[ct * 9 + j, nh].astype(np.float32)  # (128c,128n)
                out[128 * nh:128 * nh + 128] += Wtile.T @ xdq.astype(np.float32)
    out += bias.astype(np.float32)[:, None]
    return out.reshape(N_OUT, H, W)


# --------------------------------------------------------------------------
# bass kernel
# --------------------------------------------------------------------------
_CACHE = {}


def _build_nc():
    import concourse.bass as bass
    import concourse.bacc as bacc
    import concourse.mybir as mybir
    from concourse import tile

    f32, f16 = mybir.dt.float32, mybir.dt.float16
    i16 = mybir.dt.int16
    ALU = mybir.AluOpType
    ACTF = mybir.ActivationFunctionType

    nc = bacc.Bacc("TRN2", target_bir_lowering=False, debug=False)
    xpad_d = nc.dram_tensor("xpad", (NCT, 128, NPW), f16, kind="ExternalInput")
    w_d = nc.dram_tensor("wt", (128, NCT * 9 * NH * 128), f16, kind="ExternalInput")
    e_d = nc.dram_tensor("et", (36, NCT * 9 * 128), f16, kind="ExternalInput")
    b_d = nc.dram_tensor("bias", (128, NH), f32, kind="ExternalInput")
    if USE_CORR:
        wc_d = nc.dram_tensor("wcorr", (NG, 2 * NH * 128), f16, kind="ExternalInput")
    y_d = nc.dram_tensor("y", (NH, 128, L), f32, kind="ExternalOutput")

    es = ExitStack()
    with tile.TileContext(nc) as tc:
        pc = es.enter_context(tc.tile_pool(name="consts", bufs=1))
        pst = es.enter_context(tc.tile_pool(name="staging", bufs=1))
        psb = es.enter_context(tc.tile_pool(name="scalesb", bufs=1))
        pw = es.enter_context(tc.tile_pool(name="work", bufs=1))
        pyo = es.enter_context(tc.tile_pool(name="yout", bufs=2))
        pps = es.enter_context(tc.tile_pool(name="scaleps", bufs=1, space="PSUM"))
        py0 = es.enter_context(tc.tile_pool(name="yps0", bufs=1, space="PSUM"))
        py1 = es.enter_context(tc.tile_pool(name="yps1", bufs=1, space="PSUM"))
        pfm = es.enter_context(tc.tile_pool(name="f32tmp", bufs=1))
        pa = tc.alloc_tile_pool(name="arrays", bufs=1)
        pT = tc.alloc_tile_pool(name="Tstage", bufs=1)

        # ---- load constants -------------------------------------------------
        xp_sb = [pc.tile([128, AW], f16, tag=f"xp{ct}", name=f"xp{ct}")
                 for ct in range(NCT)]
        half = NPW // 2
        for ct in range(NCT):
            nc.sync.dma_start(out=xp_sb[ct][:, 0:half],
                              in_=xpad_d.ap()[ct, :, 0:half])
            nc.scalar.dma_start(out=xp_sb[ct][:, half:NPW],
                                in_=xpad_d.ap()[ct, :, half:NPW])
            nc.vector.memset(xp_sb[ct][:, NPW:AW], 0.0)
        e_sb = pc.tile([36, NCT * 9 * 128], f16, tag="esb")
        nc.sync.dma_start(out=e_sb[:], in_=e_d.ap())
        bias_sb = pc.tile([128, NH], f32, tag="bsb")
        nc.sync.dma_start(out=bias_sb[:], in_=b_d.ap())
        w_sb = pc.tile([128, NCT * 9 * NH * 128], f16, tag="wsb")
        nc.scalar.dma_start(out=w_sb[:], in_=w_d.ap())

        # ---- stage 1: image-domain max algebra (fp16, 58-layout) ------------
        # valid ranges: A [0:AW], H2 [0:AW-1], H3 [0:AW-2],
        #               V2 [0:AW-2-58], V3 [0:AW-2-116]
        arrs = {}
        for name in ARR_NAMES:
            arrs[name] = [pa.tile([128, AW], f16, tag=f"{name}{ct}", name=f"{name}{ct}")
                          for ct in range(NCT)]
        for ct in range(NCT):
            A, H2a, H3a, V2a, V3a = (arrs[n][ct] for n in ARR_NAMES)
            nc.scalar.activation(A[:], xp_sb[ct][:], ACTF.Abs)
            nc.vector.tensor_tensor(H2a[:, 0:AW - 1], A[:, 0:AW - 1],
                                    A[:, 1:AW], op=ALU.max)
            nc.vector.tensor_tensor(H3a[:, 0:AW - 2], H2a[:, 0:AW - 2],
                                    A[:, 2:AW], op=ALU.max)
            nc.vector.tensor_tensor(V2a[:, 0:AW - 2 - HP], H3a[:, 0:AW - 2 - HP],
                                    H3a[:, HP:AW - 2], op=ALU.max)
            nc.vector.tensor_tensor(V3a[:, 0:AW - 2 - 2 * HP], V2a[:, 0:AW - 2 - 2 * HP],
                                    H3a[:, 2 * HP:AW - 2], op=ALU.max)

        # ---- stage 2+3: granule gather + max tree + scales, in two column
        # bands.  Band 0 (chunks 0-3) runs to completion up front; band 1's
        # gathers are enqueued immediately but its DVE compute is deferred
        # and interleaved into the early main loop, overlapping the DMA.
        BND = ((0, 1856), (1856, NPW))
        BW0 = 1856
        m58 = pst.tile([NG, BW0], f16, tag="m58")
        rs58 = psb.tile([NG, 2 * NPW], f16, tag="rs58")
        rs58b = psb.tile([36, 2 * NPW], f16, tag="rs58b")
        rs_src = [rs58, rs58b]

        gq = [0]

        def band_ops(bi):
            """Thunks (in emission order): interleaved gather rounds and
            max-merges, then clamp + scales, for column band bi."""
            b0, b1 = BND[bi]
            bw = b1 - b0
            Ts = {}
            ops = []

            def gather(i, lo, hi, bi=bi, b0=b0, b1=b1):
                if i not in Ts:
                    Ts[i] = pT.tile([NG, b1 - b0], f16, tag=f"T{bi}",
                                    name=f"T{bi}_{i}", bufs=2)
                T_i = Ts[i]
                for gp in range(lo, hi):
                    cpp, arr, da, db = TERMS[gp][i]
                    off = HP * da + db
                    for hf in range(NCT):
                        src = arrs[arr][hf][cpp:cpp + 97:32, off + b0:off + b1]
                        dst = T_i[36 * hf + gp:36 * hf + gp + 28:9]
                        eng = nc.sync if gq[0] % 2 == 0 else nc.scalar
                        gq[0] += 1
                        eng.dma_start(out=dst, in_=src)

            def merge(i, bw=bw):
                in0 = Ts[0] if i == 1 else m58
                nc.vector.tensor_tensor(m58[:, 0:bw], in0[:, 0:bw],
                                        Ts[i][:, 0:bw], op=ALU.max)

            ops.append(lambda: gather(0, 0, 9))
            ops.append(lambda: gather(1, 0, 9))
            for i in range(2, 6):
                ops.append(lambda i=i: merge(i - 1))
                ops.append(lambda i=i: gather(i, 0, 5))
                ops.append(lambda i=i: gather(i, 5, 9))
            ops.append(lambda: merge(5))
            ops.append(lambda bw=bw: nc.vector.tensor_scalar(
                m58[:, 0:bw], m58[:, 0:bw], float(M_CLAMP), None, op0=ALU.max))
            # reciprocal in two half-band passes (f32 scratch is half-band)
            hw_ = bw // 2
            for s0, s1 in ((0, hw_), (hw_, bw)):
                def rcp_pass(s0=s0, s1=s1, b0=b0):
                    m32 = pfm.tile([NG, BW0 // 2], f32, tag="m32")
                    rcp32 = pfm.tile([NG, BW0 // 2], f32, tag="rcp32")
                    n = s1 - s0
                    nc.vector.tensor_copy(m32[:, 0:n], m58[:, s0:s1])
                    nc.vector.reciprocal_approx_fast(out=rcp32[:, 0:n],
                                                     in_=m32[:, 0:n])
                    nc.vector.tensor_scalar(rs58[:, b0 + s0:b0 + s1],
                                            rcp32[:, 0:n], float(MAXQ), None,
                                            op0=ALU.mult)
                    nc.vector.tensor_scalar(rs58[:, NPW + b0 + s0:NPW + b0 + s1],
                                            m58[:, s0:s1], float(1.0 / MAXQ),
                                            None, op0=ALU.mult)
                ops.append(rcp_pass)
            def bcast(b0=b0, b1=b1):
                nc.sync.dma_start(out=rs58b[:, b0:b1], in_=rs58[36:72, b0:b1])
                nc.scalar.dma_start(out=rs58b[:, NPW + b0:NPW + b1],
                                    in_=rs58[36:72, NPW + b0:NPW + b1])
            ops.append(bcast)
            return ops

        for op in band_ops(0):
            op()
        deferred = band_ops(1)
        import os as _os
        released = [False]
        if _os.environ.get("BAND_INLINE"):
            for op in deferred:
                op()
            deferred = []
            pT.release()
            pa.release()
            released[0] = True

        # ---- stage 4: main loop (software-pipelined, skew 1) ----------------
        segments = [(ch, ct) for ch in range(NCHUNK) for ct in range(NCT)]
        seg_rsb = {}
        seg_yps = {}

        NB = 3                      # rotating scale buffers per segment
        patch_rr = [0]              # round-robin dma issue engine

        def emit_base(si):
            """Scale-tile base builds (j=0..NB-1) for segment si."""
            ch, ct = segments[si]
            off = HP * ROWS * ch
            src = rs_src[ct]
            rsb = []
            for jb in range(NB):
                e_ap = e_sb[:, 128 * (ct * 9 + jb):128 * (ct * 9 + jb) + 128]
                rps = pps.tile([128, CW], f32, tag="rps", name="rps", bufs=2)
                sps = pps.tile([128, CW], f32, tag="sps", name="sps", bufs=2)
                nc.tensor.matmul(rps[:], e_ap, src[0:36, off:off + CW],
                                 start=True, stop=True)
                nc.tensor.matmul(sps[:], e_ap, src[0:36, NPW + off:NPW + off + CW],
                                 start=True, stop=True)
                rt = psb.tile([128, 2 * CW], f16, tag=f"rsb{ct}{jb}",
                              name=f"rsb{ct}{jb}", bufs=2)
                nc.scalar.copy(rt[:, 0:CW], rps[:])
                nc.scalar.copy(rt[:, CW:2 * CW], sps[:])
                rsb.append(rt)
            seg_rsb[si] = rsb

        def emit_unit(si, j):
            """Unit j of segment si: quantize chain, matmuls, patch."""
            ch, ct = segments[si]
            h0 = ROWS * ch
            off = HP * h0
            idx = ct * 9 + j
            buf = seg_rsb[si][j % NB]
            dh, dw = divmod(j, 3)
            xv = xp_sb[ct][:, HP * (h0 + dh) + dw:HP * (h0 + dh) + dw + CW]
            t16 = pw.tile([128, CW], f16, tag="t", bufs=3)
            qi = pw.tile([128, CW], i16, tag="qi", bufs=3)
            xdq = pw.tile([128, CW], f16, tag="x", bufs=3)
            nc.vector.tensor_tensor(t16[:], xv, buf[:, 0:CW], op=ALU.mult)
            nc.vector.tensor_copy(qi[:], t16[:])       # RNE round via i16 cast
            nc.vector.tensor_tensor(xdq[:], qi[:], buf[:, CW:2 * CW],
                                    op=ALU.mult)
            xdq_v = xdq.rearrange("p (a b) -> p a b", b=WP)[:, :, 0:W]
            yps = seg_yps[si // NCT]
            for nh in range(NH):
                wsl = w_sb[:, (idx * NH + nh) * 128:(idx * NH + nh + 1) * 128]
                nc.tensor.matmul(yps[nh][:], wsl, xdq_v,
                                 start=(idx == 0), stop=(idx == NCT * 9 - 1))
            # patch scale buffer (j % NB) toward scales of j+NB
            if j + NB <= 8:
                src = rs_src[ct]
                for jj in range(j + 1, j + NB + 1):
                    eng = nc.sync if patch_rr[0] % 2 == 0 else nc.scalar
                    patch_rr[0] += 1
                    p0, gl = crossings(jj)
                    sview = (src.rearrange("p (s c) -> p s c", s=2)
                             [gl:gl + 28:9, :, off:off + CW])
                    dview = (buf.rearrange("p (s c) -> p s c", s=2)
                             [p0:p0 + 97:32])
                    eng.dma_start(out=dview, in_=sview)

        emit_base(0)
        emit_base(1)
        for si, (ch, ct) in enumerate(segments):
            if ct == 0:
                seg_yps[ch] = [py0.tile([128, CHUNK], f32, tag="y0", name="y0",
                                        bufs=2),
                               py1.tile([128, CHUNK], f32, tag="y1", name="y1",
                                        bufs=2)]
            for j in range(9):
                emit_unit(si, j)
                if deferred:
                    deferred.pop(0)()
                    if not deferred and _os.environ.get("LATE_RELEASE"):
                        pT.release()
                        pa.release()
                        released[0] = True
                if j == 4 and si + 2 < len(segments):
                    emit_base(si + 2)
            if ct == 1:
                lsl = slice(CHUNK * ch, CHUNK * (ch + 1))
                yps = seg_yps[ch]
                for nh in range(NH):
                    ysb = pyo.tile([128, CHUNK], f32, tag=f"ysb{nh}")
                    nc.scalar.activation(ysb[:], yps[nh][:], ACTF.Identity,
                                         bias=bias_sb[:, nh:nh + 1], scale=1.0)
                    nc.sync.dma_start(out=y_d.ap()[nh, :, lsl], in_=ysb[:])
        if not released[0]:
            pT.release()
            pa.release()
        es.close()
    nc.compile()
    return nc


def build_inmaps(input, weight, bias):
    """FULL inputs -> list of 8 per-core input dicts."""
    input = np.asarray(input, np.float32)
    weight = np.asarray(weight, np.float32)
    bias = np.asarray(bias, np.float32)
    wdq = quantize_weight_host(weight)
    Wt = np.ascontiguousarray(np.transpose(pack_weights(wdq), (2, 0, 1, 3))
                              ).reshape(128, NCT * 9 * NH * 128)
    E = np.ascontiguousarray(np.transpose(build_E(), (1, 0, 2))).reshape(36, NCT * 9 * 128)
    b = np.ascontiguousarray(bias.reshape(NH, 128).T).astype(np.float32)
    base = {"wt": Wt, "et": E, "bias": b}
    if USE_CORR:
        wdq16 = wdq.astype(np.float16).astype(np.float64)
        base["wcorr"] = build_wcorr(wdq16).reshape(NG, 2 * NH * 128)
    return [dict(base, xpad=pad_image(input[bi])) for bi in range(input.shape[0])]


def kernel(input, weight, bias):
    input = np.asarray(input, np.float32)
    B = input.shape[0]
    assert B == 8 and input.shape[1:] == (C_IN, H, W)

    from concourse import bass_utils

    if "nc" not in _CACHE:
        _CACHE["nc"] = _build_nc()
    nc = _CACHE["nc"]

    in_maps = build_inmaps(input, weight, bias)
    res = bass_utils.run_bass_kernel_spmd(nc, in_maps, core_ids=list(range(B)))
    out = np.stack([r["y"].reshape(N_OUT, H, W) for r in res.results])
    return out.astype(np.float32)


if __name__ == "__main__":
    pass
